# revision 26
# baseline (speedup 1.0000x reference)
"""Trainium2 Bass kernel for nn_EnsemblesWithMessagePassing.

Sharding: data-parallel over token positions (N=512 across 8 cores, 64
positions x B=2 = 128 (b,n) tokens per core). The voting attention is
per-position over the M=16 messages, so no collectives are needed; each
core holds all members' weights and computes all 8 members for its slice.

Structural choices (engine-balance driven; DVE is the bottleneck):
  * The member Linear feeds ONLY the kv projection, so (x@Wl^T)@wkv is
    fused on CPU into x@(Wl^T@wkv): no Linear matmuls, no transposes,
    no bias rank-1s, same DMA bytes.
  * Messages are stored m-interleaved (2l = token msg, 2l+1 = Linear-out
    msg of member l) so each member's two sims run as ONE product op +
    ONE 2x add-tree (tensor_reduce is 1x-only on DVE; tensor_tensor
    bf16 runs 2x).
  * Per member the PE emits k_tok, k_out, v_tok, v_out (k's first) so
    k copies/sims never wait behind v copies; k/v weight halves are
    separate DRAM tensors so the k half of a member's fused weight
    lands first (contiguous 8KB/partition rows, 128 descriptors).
  * Softmax skips max-subtraction (logits bounded ~|3.5|), which makes
    exp(logits) computable per m-half: the o-contribution of messages
    0..7 overlaps the second half of the kv loop, halving the DVE tail.
  * q stays RAW; the rms scale rides the combined logit scale
    rk[t,l,h,m] = rs[t,l]*krinv[t,h,m]. Gates fold into the softmax
    normalizer and multiply o once per member.
  * Everything bf16 except logits/stats; output written bf16, upcast on
    CPU.

b_net is zero in the reference setup; a bias-capable variant (rank-1
matmuls of b@wkv into the out-message PSUM groups) is compiled only if
a nonzero b_net shows up.
"""
import sys

for _p in ("/opt/trn_rl_repo", "/root/.axon_site/_ro/trn_rl_repo"):
    if _p not in sys.path:
        sys.path.insert(0, _p)

try:  # NTFF profile hook glue (only needed if BASS_TRACE is set externally)
    import antenv.axon_hooks  # noqa: F401
except Exception:
    pass

from contextlib import ExitStack

import numpy as np

import concourse.tile as tile
from concourse import bacc, mybir
from concourse import bass_utils
from concourse.masks import make_identity

f32 = mybir.dt.float32
bf16 = mybir.dt.bfloat16
AF = mybir.ActivationFunctionType
AL = mybir.AluOpType
AX = mybir.AxisListType

# problem shape
L, B, N, D = 8, 2, 512, 1024
H, DH = 8, 64
INNER = H * DH          # 512
M = 2 * L               # 16 messages
SCALE = DH ** -0.5
EPS = float(np.finfo(np.float32).eps)

NCORES = 8
NSL = N // NCORES       # 64 positions per core per batch row
T = B * NSL             # 128 tokens per core
LT = L * T              # 1024
DT = D // 128           # 8 d-tiles
IT = INNER // 128       # 4 inner-tiles
MH = M // 2             # half the messages (per softmax-split half)

GP_SIM = (1, 4, 6)      # members whose sim pair runs on GPSIMD (DVE offload)
GP_O = [0, 1]           # members whose o-reduction runs on GPSIMD

_NC_CACHE = {}


def _build(has_bias=False):
    nc = bacc.Bacc("TRN2", target_bir_lowering=False, debug=False,
                   enable_asserts=False, num_devices=NCORES)

    xTb_d = nc.dram_tensor("xTb", [128, DT, LT], bf16, kind="ExternalInput").ap()
    wkvk_d = nc.dram_tensor("wkvk", [128, DT, INNER], bf16,
                            kind="ExternalInput").ap()
    wkvv_d = nc.dram_tensor("wkvv", [128, DT, INNER], bf16,
                            kind="ExternalInput").ap()
    wfk_d = nc.dram_tensor("wfk", [L, 128, DT, INNER], bf16,
                           kind="ExternalInput").ap()
    wfv_d = nc.dram_tensor("wfv", [L, 128, DT, INNER], bf16,
                           kind="ExternalInput").ap()
    wq_d = nc.dram_tensor("wqT", [128, DT, INNER], bf16, kind="ExternalInput").ap()
    wg_d = nc.dram_tensor("wgT", [128, DT, H], bf16, kind="ExternalInput").ap()
    wout_d = nc.dram_tensor("woutT", [128, IT, D], bf16, kind="ExternalInput").ap()
    bkv_d = nc.dram_tensor("bkv", [1, L, 2 * INNER], bf16,
                           kind="ExternalInput").ap()
    out_d = nc.dram_tensor("out", [L, T, D], bf16, kind="ExternalOutput").ap()

    with tile.TileContext(nc) as tc, ExitStack() as ctx:
        pc = ctx.enter_context(tc.tile_pool(name="const", bufs=1))
        pa = ctx.enter_context(tc.tile_pool(name="attp", bufs=1))
        po = ctx.enter_context(tc.tile_pool(name="oscr", bufs=2))
        pg = ctx.enter_context(tc.tile_pool(name="gscr", bufs=1))
        pmm = ctx.enter_context(tc.tile_pool(name="psmm", bufs=6, space="PSUM"))

        # ---- constants ----
        ident_b = pc.tile([128, 128], bf16, tag="ident_b")
        make_identity(nc, ident_b)
        ones_f = pc.tile([1, 2], f32, tag="ones_f")
        nc.vector.memset(ones_f[:], 1.0)
        onesc = pc.tile([128, 2], bf16, tag="onesc")
        nc.vector.memset(onesc[:], 1.0)
        eps_c = pc.tile([128, 1], f32, tag="eps")
        nc.vector.memset(eps_c[:], EPS)
        if has_bias:
            onesb = pc.tile([1, 128], bf16, tag="onesb")
            nc.vector.memset(onesb[:], 1.0)
            bkv = pc.tile([1, L, 2 * INNER], bf16, tag="bkv")
            nc.sync.dma_start(bkv[:], bkv_d[:])

        # whole-kernel attention state (m-interleaved: 2l=token, 2l+1=out)
        k_all = pa.tile([128, M, INNER], bf16, tag="k_all")     # RAW k
        v_allT = pa.tile([128, H, DH, M], bf16, tag="v_allT")   # m innermost
        q_all = pa.tile([128, L, INNER], bf16, tag="q_all")     # RAW q (no rs)
        g_all = pa.tile([128, L, H], f32, tag="g_all")
        kss = pa.tile([128, M, H], f32, tag="kss")              # m-major
        krinv = pa.tile([128, M, H], f32, tag="krinv")
        sim_all = pa.tile([128, L, H, M], f32, tag="sim_all")   # RAW q.k
        pl_all = pa.tile([128, L, H, M], bf16, tag="pl_all")
        o_acc = pa.tile([128, L, INNER], bf16, tag="o_acc")
        rs_tok = pa.tile([128, L], f32, tag="rs_tok")
        rk = pa.tile([128, L, H, M], f32, tag="rk")

        with ExitStack() as ctx_b:
            pb = ctx_b.enter_context(tc.tile_pool(name="bigact", bufs=1))
            ps_ = ctx_b.enter_context(tc.tile_pool(name="scr", bufs=2))
            psm = ctx_b.enter_context(tc.tile_pool(name="pssm", bufs=2,
                                                   space="PSUM"))

            xTb = pb.tile([128, DT, LT], bf16, tag="xTb")
            nc.sync.dma_start(xTb[:], xTb_d[:])
            wq = pb.tile([128, DT, INNER], bf16, tag="wq")
            nc.sync.dma_start(wq[:], wq_d[:])
            wkvk = pb.tile([128, DT, INNER], bf16, tag="wkvk")
            nc.sync.dma_start(wkvk[:], wkvk_d[:])
            wkvv = pb.tile([128, DT, INNER], bf16, tag="wkvv")
            nc.sync.dma_start(wkvv[:], wkvv_d[:])
            wg = pb.tile([128, DT, H], bf16, tag="wg")
            nc.sync.dma_start(wg[:], wg_d[:])

            def mm_sweep(ps, wsrc, l, nrows=INNER, extra=None):
                for d in range(DT):
                    nc.tensor.matmul(ps[:, 0:nrows], xTb[:, d, l * T:(l + 1) * T],
                                     wsrc[:, d, 0:nrows], start=(d == 0),
                                     stop=(d == DT - 1) and extra is None)
                if extra is not None:
                    nc.tensor.matmul(ps[:, 0:nrows], onesb[0:1, :], extra,
                                     start=False, stop=True)

            # ---- RAW queries first: the sim pipeline must start early ----
            with nc.named_scope("qg"):
                for l in range(L):
                    qps = pmm.tile([128, INNER], f32, tag="mm")
                    mm_sweep(qps, wq, l)
                    nc.scalar.copy(q_all[:, l], qps[:])

            # ---- token rms stats (token-major; rs folds into logits) ----
            with nc.named_scope("stats"), ExitStack() as ctx_s:
                pst = ctx_s.enter_context(tc.tile_pool(name="statp", bufs=1))
                sq = pst.tile([128, DT, LT], bf16, tag="sq")
                nc.vector.tensor_tensor(
                    sq.rearrange("p d t -> p (d t)"),
                    xTb.rearrange("p d t -> p (d t)"),
                    xTb.rearrange("p d t -> p (d t)"), AL.mult)
                ssq_row = pst.tile([1, LT], f32, tag="ssq_row")
                for c in range(2):
                    ssps = psm.tile([2, 512], f32, tag="sm")
                    for d in range(DT):
                        nc.tensor.matmul(ssps[:], onesc[:],
                                         sq[:, d, c * 512:(c + 1) * 512],
                                         start=(d == 0), stop=(d == DT - 1))
                    nc.scalar.copy(ssq_row[0:1, c * 512:(c + 1) * 512],
                                   ssps[0:1, :])
                rsps = psm.tile([128, L], f32, tag="sm")
                for l in range(L):
                    nc.tensor.matmul(rsps[:, l:l + 1],
                                     ssq_row[0:1, l * T:(l + 1) * T],
                                     ones_f[0:1, 0:1], start=True, stop=True)
                rms_tok = pst.tile([128, L], f32, tag="rms_tok")
                nc.scalar.activation(rms_tok[:], rsps[:], AF.Sqrt,
                                     scale=1.0 / D, bias=eps_c[:, 0:1])
                nc.vector.reciprocal(rs_tok[:], rms_tok[:])

            def k_post(m):
                ksq = pg.tile([128, INNER], f32, tag="ksq")
                nc.scalar.square(ksq[:], k_all[:, m])
                nc.vector.tensor_reduce(
                    kss[:, m], ksq.rearrange("p (h d) -> p h d", d=DH),
                    axis=AX.X, op=AL.add)

            def sim_pair(l, eng):
                # both messages of member l in one product + one 2x add-tree;
                # eng is nc.vector or nc.gpsimd (DVE offload)
                pool = ps_ if eng is nc.vector else pg
                scr = pool.tile([128, L, 2, INNER], bf16,
                                tag="scr" if eng is nc.vector else "scrg")
                m0 = 2 * l
                eng.tensor_tensor(
                    scr[:],
                    q_all[:, :, None].to_broadcast([128, L, 2, INNER]),
                    k_all[:, None, m0:m0 + 2].to_broadcast([128, L, 2, INNER]),
                    AL.mult)
                sv = scr.rearrange("p l m (h d) -> p (l m) h d", d=DH)
                w = DH // 2
                while w >= 1:
                    eng.tensor_tensor(sv[:, :, :, 0:w], sv[:, :, :, 0:w],
                                      sv[:, :, :, w:2 * w], AL.add)
                    w //= 2
                # deposit [p, l, m2, h] into sim_all's [p, l, h, m] layout
                eng.tensor_copy(
                    sim_all[:, :, :, m0:m0 + 2].rearrange("p l h m -> p l m h"),
                    scr.rearrange("p l m (h d) -> p l m h d", d=DH)[:, :, :, :, 0])

            def half_softmax_o(half):
                # exp over messages [half*MH, (half+1)*MH) and their o part
                ms = slice(half * MH, (half + 1) * MH)
                krm = po.tile([128, MH, H], f32, tag="krm")
                nc.scalar.activation(krm.rearrange("p m h -> p (m h)"),
                                     kss[:, ms].rearrange("p m h -> p (m h)"),
                                     AF.Sqrt, scale=1.0 / DH, bias=eps_c[:, 0:1])
                nc.vector.reciprocal(
                    krinv[:, ms].rearrange("p m h -> p (m h)"),
                    krm.rearrange("p m h -> p (m h)"))
                nc.vector.tensor_tensor(
                    rk[:, :, :, ms],
                    rs_tok[:, :, None, None].to_broadcast([128, L, H, MH]),
                    krinv[:, ms].rearrange("p m h -> p h m")[:, None]
                    .to_broadcast([128, L, H, MH]), AL.mult)
                nc.vector.tensor_tensor(sim_all[:, :, :, ms], sim_all[:, :, :, ms],
                                        rk[:, :, :, ms], AL.mult)
                nc.scalar.activation(
                    pl_all[:, :, :, ms].rearrange("p l h m -> p (l h) m"),
                    sim_all[:, :, :, ms].rearrange("p l h m -> p (l h) m"),
                    AF.Exp)
                for l in GP_O + [x for x in range(L) if x not in GP_O]:
                    eng = nc.gpsimd if l in GP_O else nc.vector
                    prod = (pg if l in GP_O else po).tile(
                        [128, H, DH, MH], bf16,
                        tag="prodg" if l in GP_O else "prod")
                    eng.tensor_tensor(
                        prod[:],
                        pl_all[:, l, :, None, ms].to_broadcast([128, H, DH, MH]),
                        v_allT[:, :, :, ms], AL.mult)
                    for w in (MH // 2, MH // 4):
                        eng.tensor_tensor(
                            prod[:, :, :, 0:w], prod[:, :, :, 0:w],
                            prod[:, :, :, w:2 * w], AL.add)
                    ov = o_acc[:, l].rearrange("p (h d) -> p h d", d=DH)
                    if half == 0:
                        eng.tensor_tensor(ov, prod[:, :, :, 0],
                                          prod[:, :, :, 1], AL.add)
                    else:
                        eng.tensor_tensor(prod[:, :, :, 0], prod[:, :, :, 0],
                                          prod[:, :, :, 1], AL.add)
                        eng.tensor_tensor(ov, ov, prod[:, :, :, 0], AL.add)

            # ---- kv + sims, with first-half softmax/o overlapped ----
            with ExitStack() as ctx_l, nc.named_scope("linkv"):
                plw = ctx_l.enter_context(tc.tile_pool(name="linw", bufs=2))
                wfk_t, wfv_t = [], []

                def fetch(j):
                    tk = plw.tile([128, DT, INNER], bf16, tag="wfk")
                    nc.sync.dma_start(tk[:], wfk_d[j])
                    tv = plw.tile([128, DT, INNER], bf16, tag="wfv")
                    nc.sync.dma_start(tv[:], wfv_d[j])
                    wfk_t.append(tk)
                    wfv_t.append(tv)

                fetch(0)
                fetch(1)
                for l in range(L):
                    m0 = 2 * l
                    bk = bv = None
                    if has_bias:
                        bk = bkv[0:1, l, 0:INNER]
                        bv = bkv[0:1, l, INNER:2 * INNER]
                    kt = pmm.tile([128, INNER], f32, tag="mm")
                    mm_sweep(kt, wkvk, l)
                    nc.scalar.copy(k_all[:, m0], kt[:])
                    ko = pmm.tile([128, INNER], f32, tag="mm")
                    mm_sweep(ko, wfk_t[l], l, extra=bk)
                    nc.scalar.copy(k_all[:, m0 + 1], ko[:])
                    k_post(m0)
                    k_post(m0 + 1)
                    sim_pair(l, nc.gpsimd if l in GP_SIM else nc.vector)
                    vt = pmm.tile([128, INNER], f32, tag="mm")
                    mm_sweep(vt, wkvv, l)
                    nc.scalar.copy(v_allT[:, :, :, m0],
                                   vt.rearrange("p (h d) -> p h d", d=DH))
                    vo = pmm.tile([128, INNER], f32, tag="mm")
                    mm_sweep(vo, wfv_t[l], l, extra=bv)
                    nc.scalar.copy(v_allT[:, :, :, m0 + 1],
                                   vo.rearrange("p (h d) -> p h d", d=DH))
                    if l + 2 < L:
                        fetch(l + 2)
                    if l == L // 2 - 1:
                        half_softmax_o(0)   # overlaps members 4..7

                # gates last (PE is free here; g needed only for rg below)
                for l in range(L):
                    gps = psm.tile([128, L], f32, tag="sm")
                    mm_sweep(gps, wg, l, nrows=H)
                    nc.scalar.activation(g_all[:, l], gps[:, 0:H], AF.Sigmoid,
                                         scale=rs_tok[:, l:l + 1])

        # ---- second softmax half + normalize + pooled ----
        with ExitStack() as ctx_e:
            pe = ctx_e.enter_context(tc.tile_pool(name="outp", bufs=1))
            pes = ctx_e.enter_context(tc.tile_pool(name="outs", bufs=2))
            ptp = ctx_e.enter_context(tc.tile_pool(name="pstp", bufs=2,
                                                   space="PSUM"))
            wout = pe.tile([128, IT, D], bf16, tag="wout")
            nc.sync.dma_start(wout[:], wout_d[:])
            with nc.named_scope("attn"):
                half_softmax_o(1)
                sm_all = pe.tile([128, L, H], f32, tag="sm_all")
                nc.vector.tensor_reduce(sm_all[:], pl_all[:], axis=AX.X, op=AL.add)
                rgf = pe.tile([128, L, H], f32, tag="rgf")
                nc.vector.reciprocal(rgf.rearrange("p l h -> p (l h)"),
                                     sm_all.rearrange("p l h -> p (l h)"))
                rg = pe.tile([128, L, H], bf16, tag="rg")
                nc.vector.tensor_tensor(rg[:], rgf[:], g_all[:], AL.mult)
                # gpsimd-owned members' o finishes last: schedule them at the end
                o_fin = pe.tile([128, L, INNER], bf16, tag="o_fin")
                tail_order = [x for x in range(L) if x not in GP_O] + GP_O
                for l in tail_order:
                    nc.vector.tensor_tensor(
                        o_fin[:, l].rearrange("p (h d) -> p h d", d=DH),
                        o_acc[:, l].rearrange("p (h d) -> p h d", d=DH),
                        rg[:, l, :, None].to_broadcast([128, H, DH]), AL.mult)
                for l in tail_order:
                    oT = pes.tile([128, IT, T], bf16, tag="oT")
                    for it in range(IT):
                        tps = ptp.tile([128, 128], bf16, tag="tp")
                        nc.tensor.transpose(
                            tps[:], o_fin[:, l, it * 128:(it + 1) * 128],
                            ident_b[:])
                        nc.scalar.copy(oT[:, it], tps[:])
                    pout = pes.tile([128, D], bf16, tag="pout")
                    for oc in range(2):
                        ps = pmm.tile([128, INNER], f32, tag="mm")
                        for it in range(IT):
                            nc.tensor.matmul(ps[:], oT[:, it],
                                             wout[:, it, oc * 512:(oc + 1) * 512],
                                             start=(it == 0), stop=(it == IT - 1))
                        nc.scalar.copy(pout[:, oc * 512:(oc + 1) * 512], ps[:])
                    nc.sync.dma_start(out_d[l][:], pout[:])

    nc.compile()
    return nc


def get_nc(has_bias=False):
    if has_bias not in _NC_CACHE:
        _NC_CACHE[has_bias] = _build(has_bias)
    return _NC_CACHE[has_bias]


def _feat_major(w):
    """[D, F] -> [128, DT_w, F] with the contraction dim on partitions."""
    dt_w = w.shape[0] // 128
    return np.ascontiguousarray(
        w.reshape(dt_w, 128, w.shape[1]).transpose(1, 0, 2))


def prep_weights(w_net, b_net, norm_w, wq, wkv, knorm_w, wg, wout):
    import ml_dtypes
    bf = ml_dtypes.bfloat16

    wf = np.matmul(w_net.transpose(0, 2, 1), wkv[None])          # [L, D, 2I]
    colscale = (np.tile(knorm_w, H) * SCALE).astype(np.float32)
    wq2 = norm_w[:, None] * wq * colscale[None, :]
    out = dict(
        wkvk=_feat_major(wkv[:, :INNER]).astype(bf),
        wkvv=_feat_major(wkv[:, INNER:]).astype(bf),
        wfk=np.stack([_feat_major(wf[l, :, :INNER]) for l in range(L)])
        .astype(bf),
        wfv=np.stack([_feat_major(wf[l, :, INNER:]) for l in range(L)])
        .astype(bf),
        wqT=_feat_major(wq2).astype(bf),
        wgT=_feat_major(norm_w[:, None] * wg).astype(bf),
        woutT=_feat_major(wout).astype(bf),
        bkv=np.ascontiguousarray((b_net @ wkv)[None]).astype(bf),
    )
    return out


def prep_core_x(tokens, c):
    import ml_dtypes
    xs = tokens[:, :, c * NSL:(c + 1) * NSL, :].reshape(L, T, D)
    xT = xs.reshape(L, T, DT, 128).transpose(3, 2, 0, 1).reshape(128, DT, LT)
    return np.ascontiguousarray(xT).astype(ml_dtypes.bfloat16)


def make_in_maps(tokens, w_net, b_net, norm_w, wq, wkv, knorm_w, wg, wout):
    shared = prep_weights(np.asarray(w_net, np.float32), np.asarray(b_net, np.float32),
                          np.asarray(norm_w, np.float32), np.asarray(wq, np.float32),
                          np.asarray(wkv, np.float32), np.asarray(knorm_w, np.float32),
                          np.asarray(wg, np.float32), np.asarray(wout, np.float32))
    tokens = np.asarray(tokens, np.float32)
    return [dict(shared, xTb=prep_core_x(tokens, c)) for c in range(NCORES)]


def stitch(results):
    full = np.empty((L, B, N, D), dtype=np.float32)
    for c in range(NCORES):
        full[:, :, c * NSL:(c + 1) * NSL, :] = \
            results[c]["out"].astype(np.float32).reshape(L, B, NSL, D)
    return full


def kernel(tokens, w_net, b_net, norm_w, wq, wkv, knorm_w, wg, wout):
    nc = get_nc(has_bias=bool(np.any(np.asarray(b_net))))
    in_maps = make_in_maps(tokens, w_net, b_net, norm_w, wq, wkv, knorm_w, wg, wout)
    res = bass_utils.run_bass_kernel_spmd(nc, in_maps, core_ids=list(range(NCORES)))
    return stitch(res.results)


# revision 30
# speedup vs baseline: 1.5334x; 1.5334x over previous
"""Trainium2 Bass kernel for nn_EnsemblesWithMessagePassing.

Sharding: data-parallel over token positions (N=512 across 8 cores, 64
positions x B=2 = 128 (b,n) tokens per core). The voting attention is
per-position over the M=16 messages, so no collectives are needed; each
core holds all members' weights and computes all 8 members for its slice.

Structural choices (engine-balance driven; DVE is the bottleneck):
  * The member Linear feeds ONLY the kv projection, so (x@Wl^T)@wkv is
    fused on CPU into x@(Wl^T@wkv): no Linear matmuls, no transposes,
    no bias rank-1s, same DMA bytes.
  * Messages are stored m-interleaved (2l = token msg, 2l+1 = Linear-out
    msg of member l) so each member's two sims run as ONE product op +
    ONE 2x add-tree (tensor_reduce is 1x-only on DVE; tensor_tensor
    bf16 runs 2x).
  * Per member the PE emits k_tok, k_out, v_tok, v_out (k's first) so
    k copies/sims never wait behind v copies; k/v weight halves are
    separate DRAM tensors so the k half of a member's fused weight
    lands first (contiguous 8KB/partition rows, 128 descriptors).
  * Softmax skips max-subtraction (logits bounded ~|3.5|), which makes
    exp(logits) computable per m-half: the o-contribution of messages
    0..7 overlaps the second half of the kv loop, halving the DVE tail.
  * q stays RAW; the rms scale rides the combined logit scale
    rk[t,l,h,m] = rs[t,l]*krinv[t,h,m]. Gates fold into the softmax
    normalizer and multiply o once per member.
  * Everything bf16 except logits/stats; output written bf16, upcast on
    CPU.

b_net is zero in the reference setup; a bias-capable variant (rank-1
matmuls of b@wkv into the out-message PSUM groups) is compiled only if
a nonzero b_net shows up.
"""
import sys

for _p in ("/opt/trn_rl_repo", "/root/.axon_site/_ro/trn_rl_repo"):
    if _p not in sys.path:
        sys.path.insert(0, _p)

try:  # NTFF profile hook glue (only needed if BASS_TRACE is set externally)
    import antenv.axon_hooks  # noqa: F401
except Exception:
    pass

from contextlib import ExitStack

import numpy as np

import concourse.tile as tile
from concourse import bacc, mybir
from concourse import bass_utils
from concourse.masks import make_identity

f32 = mybir.dt.float32
bf16 = mybir.dt.bfloat16
AF = mybir.ActivationFunctionType
AL = mybir.AluOpType
AX = mybir.AxisListType

# problem shape
L, B, N, D = 8, 2, 512, 1024
H, DH = 8, 64
INNER = H * DH          # 512
M = 2 * L               # 16 messages
SCALE = DH ** -0.5
EPS = float(np.finfo(np.float32).eps)

NCORES = 8
NSL = N // NCORES       # 64 positions per core per batch row
T = B * NSL             # 128 tokens per core
LT = L * T              # 1024
DT = D // 128           # 8 d-tiles
IT = INNER // 128       # 4 inner-tiles
MH = M // 2             # half the messages (per softmax-split half)

GP_SIM = ()             # members whose sim pair runs on GPSIMD (measured: the
GP_O = []               # offload inflates DVE op times — keep everything on DVE)

_NC_CACHE = {}


def _build(has_bias=False):
    nc = bacc.Bacc("TRN2", target_bir_lowering=False, debug=False,
                   enable_asserts=False, num_devices=NCORES)

    xTb_d = nc.dram_tensor("xTb", [128, DT, LT], bf16, kind="ExternalInput").ap()
    wkvk_d = nc.dram_tensor("wkvk", [128, DT, INNER], bf16,
                            kind="ExternalInput").ap()
    wkvv_d = nc.dram_tensor("wkvv", [128, DT, INNER], bf16,
                            kind="ExternalInput").ap()
    wfk_d = nc.dram_tensor("wfk", [L, 128, DT, INNER], bf16,
                           kind="ExternalInput").ap()
    wfv_d = nc.dram_tensor("wfv", [L, 128, DT, INNER], bf16,
                           kind="ExternalInput").ap()
    wq_d = nc.dram_tensor("wqT", [128, DT, INNER], bf16, kind="ExternalInput").ap()
    wg_d = nc.dram_tensor("wgT", [128, DT, H], bf16, kind="ExternalInput").ap()
    wout_d = nc.dram_tensor("woutT", [128, IT, D], bf16, kind="ExternalInput").ap()
    bkv_d = nc.dram_tensor("bkv", [1, L, 2 * INNER], bf16,
                           kind="ExternalInput").ap()
    out_d = nc.dram_tensor("out", [L, T, D], bf16, kind="ExternalOutput").ap()

    with tile.TileContext(nc) as tc, ExitStack() as ctx:
        pc = ctx.enter_context(tc.tile_pool(name="const", bufs=1))
        pa = ctx.enter_context(tc.tile_pool(name="attp", bufs=1))
        po = ctx.enter_context(tc.tile_pool(name="oscr", bufs=2))
        pg = ctx.enter_context(tc.tile_pool(name="gscr", bufs=1))
        pmm = ctx.enter_context(tc.tile_pool(name="psmm", bufs=6, space="PSUM"))

        # ---- constants ----
        ident_b = pc.tile([128, 128], bf16, tag="ident_b")
        make_identity(nc, ident_b)
        ones_f = pc.tile([1, 2], f32, tag="ones_f")
        nc.vector.memset(ones_f[:], 1.0)
        onesc = pc.tile([128, 2], bf16, tag="onesc")
        nc.vector.memset(onesc[:], 1.0)
        eps_c = pc.tile([128, 1], f32, tag="eps")
        nc.vector.memset(eps_c[:], EPS)
        if has_bias:
            onesb = pc.tile([1, 128], bf16, tag="onesb")
            nc.vector.memset(onesb[:], 1.0)
            bkv = pc.tile([1, L, 2 * INNER], bf16, tag="bkv")
            nc.sync.dma_start(bkv[:], bkv_d[:])

        # whole-kernel attention state (m-interleaved: 2l=token, 2l+1=out)
        k_all = pa.tile([128, M, INNER], bf16, tag="k_all")     # RAW k
        v_allT = pa.tile([128, H, DH, M], bf16, tag="v_allT")   # m innermost
        q_all = pa.tile([128, L, INNER], bf16, tag="q_all")     # RAW q (no rs)
        g_all = pa.tile([128, L, H], f32, tag="g_all")
        kss = pa.tile([128, M, H], f32, tag="kss")              # m-major
        krinv = pa.tile([128, M, H], f32, tag="krinv")
        sim_all = pa.tile([128, L, H, M], f32, tag="sim_all")   # RAW q.k
        pl_all = pa.tile([128, L, H, M], bf16, tag="pl_all")
        o_acc = pa.tile([128, L, INNER], bf16, tag="o_acc")
        rs_tok = pa.tile([128, L], f32, tag="rs_tok")
        rk = pa.tile([128, L, H, M], f32, tag="rk")

        with ExitStack() as ctx_b:
            pb = ctx_b.enter_context(tc.tile_pool(name="bigact", bufs=1))
            ps_ = ctx_b.enter_context(tc.tile_pool(name="scr", bufs=2))
            psm = ctx_b.enter_context(tc.tile_pool(name="pssm", bufs=2,
                                                   space="PSUM"))

            xTb = pb.tile([128, DT, LT], bf16, tag="xTb")
            nc.sync.dma_start(xTb[:], xTb_d[:])
            wq = pb.tile([128, DT, INNER], bf16, tag="wq")
            nc.sync.dma_start(wq[:], wq_d[:])
            wkvk = pb.tile([128, DT, INNER], bf16, tag="wkvk")
            nc.sync.dma_start(wkvk[:], wkvk_d[:])
            wkvv = pb.tile([128, DT, INNER], bf16, tag="wkvv")
            nc.sync.dma_start(wkvv[:], wkvv_d[:])
            wg = pb.tile([128, DT, H], bf16, tag="wg")
            nc.sync.dma_start(wg[:], wg_d[:])

            def mm_sweep(ps, wsrc, l, nrows=INNER, extra=None):
                for d in range(DT):
                    nc.tensor.matmul(ps[:, 0:nrows], xTb[:, d, l * T:(l + 1) * T],
                                     wsrc[:, d, 0:nrows], start=(d == 0),
                                     stop=(d == DT - 1) and extra is None)
                if extra is not None:
                    nc.tensor.matmul(ps[:, 0:nrows], onesb[0:1, :], extra,
                                     start=False, stop=True)

            # ---- RAW queries first: the sim pipeline must start early ----
            with nc.named_scope("qg"):
                for l in range(L):
                    qps = pmm.tile([128, INNER], f32, tag="mm")
                    mm_sweep(qps, wq, l)
                    nc.scalar.copy(q_all[:, l], qps[:])

            def k_post(m):
                ksq = pg.tile([128, INNER], f32, tag="ksq")
                nc.scalar.square(ksq[:], k_all[:, m])
                nc.vector.tensor_reduce(
                    kss[:, m], ksq.rearrange("p (h d) -> p h d", d=DH),
                    axis=AX.X, op=AL.add)

            # ---- all token-message k's next (only need wkvk): the first
            # sims then wait only on the member's fused k weight DMA ----
            with nc.named_scope("ktok"):
                for l in range(L):
                    kt = pmm.tile([128, INNER], f32, tag="mm")
                    mm_sweep(kt, wkvk, l)
                    nc.scalar.copy(k_all[:, 2 * l], kt[:])
                    k_post(2 * l)

            # ---- token rms stats (token-major; rs folds into logits) ----
            with nc.named_scope("stats"), ExitStack() as ctx_s:
                pst = ctx_s.enter_context(tc.tile_pool(name="statp", bufs=1))
                sq = pst.tile([128, DT, LT], bf16, tag="sq")
                nc.vector.tensor_tensor(
                    sq.rearrange("p d t -> p (d t)"),
                    xTb.rearrange("p d t -> p (d t)"),
                    xTb.rearrange("p d t -> p (d t)"), AL.mult)
                ssq_row = pst.tile([1, LT], f32, tag="ssq_row")
                for c in range(2):
                    ssps = psm.tile([2, 512], f32, tag="sm")
                    for d in range(DT):
                        nc.tensor.matmul(ssps[:], onesc[:],
                                         sq[:, d, c * 512:(c + 1) * 512],
                                         start=(d == 0), stop=(d == DT - 1))
                    nc.scalar.copy(ssq_row[0:1, c * 512:(c + 1) * 512],
                                   ssps[0:1, :])
                rsps = psm.tile([128, L], f32, tag="sm")
                for l in range(L):
                    nc.tensor.matmul(rsps[:, l:l + 1],
                                     ssq_row[0:1, l * T:(l + 1) * T],
                                     ones_f[0:1, 0:1], start=True, stop=True)
                rms_tok = pst.tile([128, L], f32, tag="rms_tok")
                nc.scalar.activation(rms_tok[:], rsps[:], AF.Sqrt,
                                     scale=1.0 / D, bias=eps_c[:, 0:1])
                nc.vector.reciprocal(rs_tok[:], rms_tok[:])

            def sim_pair(l, eng):
                # both messages of member l in one product + one 2x add-tree;
                # eng is nc.vector or nc.gpsimd (DVE offload)
                pool = ps_ if eng is nc.vector else pg
                scr = pool.tile([128, L, 2, INNER], bf16,
                                tag="scr" if eng is nc.vector else "scrg")
                m0 = 2 * l
                eng.tensor_tensor(
                    scr[:],
                    q_all[:, :, None].to_broadcast([128, L, 2, INNER]),
                    k_all[:, None, m0:m0 + 2].to_broadcast([128, L, 2, INNER]),
                    AL.mult)
                sv = scr.rearrange("p l m (h d) -> p (l m) h d", d=DH)
                w = DH // 2
                while w >= 1:
                    eng.tensor_tensor(sv[:, :, :, 0:w], sv[:, :, :, 0:w],
                                      sv[:, :, :, w:2 * w], AL.add)
                    w //= 2
                # deposit [p, l, m2, h] into sim_all's [p, l, h, m] layout
                eng.tensor_copy(
                    sim_all[:, :, :, m0:m0 + 2].rearrange("p l h m -> p l m h"),
                    scr.rearrange("p l m (h d) -> p l m h d", d=DH)[:, :, :, :, 0])

            def half_softmax_o(half):
                # exp over messages [half*MH, (half+1)*MH) and their o part
                ms = slice(half * MH, (half + 1) * MH)
                krm = po.tile([128, MH, H], f32, tag="krm")
                nc.scalar.activation(krm.rearrange("p m h -> p (m h)"),
                                     kss[:, ms].rearrange("p m h -> p (m h)"),
                                     AF.Sqrt, scale=1.0 / DH, bias=eps_c[:, 0:1])
                nc.vector.reciprocal(
                    krinv[:, ms].rearrange("p m h -> p (m h)"),
                    krm.rearrange("p m h -> p (m h)"))
                nc.vector.tensor_tensor(
                    rk[:, :, :, ms],
                    rs_tok[:, :, None, None].to_broadcast([128, L, H, MH]),
                    krinv[:, ms].rearrange("p m h -> p h m")[:, None]
                    .to_broadcast([128, L, H, MH]), AL.mult)
                nc.vector.tensor_tensor(sim_all[:, :, :, ms], sim_all[:, :, :, ms],
                                        rk[:, :, :, ms], AL.mult)
                nc.scalar.activation(
                    pl_all[:, :, :, ms].rearrange("p l h m -> p (l h) m"),
                    sim_all[:, :, :, ms].rearrange("p l h m -> p (l h) m"),
                    AF.Exp)
                for l in GP_O + [x for x in range(L) if x not in GP_O]:
                    eng = nc.gpsimd if l in GP_O else nc.vector
                    prod = (pg if l in GP_O else po).tile(
                        [128, H, DH, MH], bf16,
                        tag="prodg" if l in GP_O else "prod")
                    eng.tensor_tensor(
                        prod[:],
                        pl_all[:, l, :, None, ms].to_broadcast([128, H, DH, MH]),
                        v_allT[:, :, :, ms], AL.mult)
                    for w in (MH // 2, MH // 4):
                        eng.tensor_tensor(
                            prod[:, :, :, 0:w], prod[:, :, :, 0:w],
                            prod[:, :, :, w:2 * w], AL.add)
                    ov = o_acc[:, l].rearrange("p (h d) -> p h d", d=DH)
                    if half == 0:
                        eng.tensor_tensor(ov, prod[:, :, :, 0],
                                          prod[:, :, :, 1], AL.add)
                    else:
                        eng.tensor_tensor(prod[:, :, :, 0], prod[:, :, :, 0],
                                          prod[:, :, :, 1], AL.add)
                        eng.tensor_tensor(ov, ov, prod[:, :, :, 0], AL.add)

            # ---- kv + sims, with first-half softmax/o overlapped ----
            with ExitStack() as ctx_l, nc.named_scope("linkv"):
                plw = ctx_l.enter_context(tc.tile_pool(name="linw", bufs=2))
                wfk_t, wfv_t = [], []

                def fetch(j):
                    tk = plw.tile([128, DT, INNER], bf16, tag="wfk")
                    nc.sync.dma_start(tk[:], wfk_d[j])
                    tv = plw.tile([128, DT, INNER], bf16, tag="wfv")
                    nc.sync.dma_start(tv[:], wfv_d[j])
                    wfk_t.append(tk)
                    wfv_t.append(tv)

                fetch(0)
                fetch(1)
                for l in range(L):
                    m0 = 2 * l
                    bk = bv = None
                    if has_bias:
                        bk = bkv[0:1, l, 0:INNER]
                        bv = bkv[0:1, l, INNER:2 * INNER]
                    ko = pmm.tile([128, INNER], f32, tag="mm")
                    mm_sweep(ko, wfk_t[l], l, extra=bk)
                    nc.scalar.copy(k_all[:, m0 + 1], ko[:])
                    k_post(m0 + 1)
                    sim_pair(l, nc.gpsimd if l in GP_SIM else nc.vector)
                    vt = pmm.tile([128, INNER], f32, tag="mm")
                    mm_sweep(vt, wkvv, l)
                    nc.scalar.copy(v_allT[:, :, :, m0],
                                   vt.rearrange("p (h d) -> p h d", d=DH))
                    vo = pmm.tile([128, INNER], f32, tag="mm")
                    mm_sweep(vo, wfv_t[l], l, extra=bv)
                    nc.scalar.copy(v_allT[:, :, :, m0 + 1],
                                   vo.rearrange("p (h d) -> p h d", d=DH))
                    if l + 2 < L:
                        fetch(l + 2)
                    if l == L // 2 - 1:
                        half_softmax_o(0)   # overlaps members 4..7

                # gates last (PE is free here; g needed only for rg below)
                for l in range(L):
                    gps = psm.tile([128, L], f32, tag="sm")
                    mm_sweep(gps, wg, l, nrows=H)
                    nc.scalar.activation(g_all[:, l], gps[:, 0:H], AF.Sigmoid,
                                         scale=rs_tok[:, l:l + 1])

        # ---- second softmax half + normalize + pooled ----
        with ExitStack() as ctx_e:
            pe = ctx_e.enter_context(tc.tile_pool(name="outp", bufs=1))
            pes = ctx_e.enter_context(tc.tile_pool(name="outs", bufs=2))
            ptp = ctx_e.enter_context(tc.tile_pool(name="pstp", bufs=2,
                                                   space="PSUM"))
            wout = pe.tile([128, IT, D], bf16, tag="wout")
            nc.sync.dma_start(wout[:], wout_d[:])
            with nc.named_scope("attn"):
                half_softmax_o(1)
                sm_all = pe.tile([128, L, H], f32, tag="sm_all")
                nc.vector.tensor_reduce(sm_all[:], pl_all[:], axis=AX.X, op=AL.add)
                rgf = pe.tile([128, L, H], f32, tag="rgf")
                nc.vector.reciprocal(rgf.rearrange("p l h -> p (l h)"),
                                     sm_all.rearrange("p l h -> p (l h)"))
                rg = pe.tile([128, L, H], bf16, tag="rg")
                nc.vector.tensor_tensor(rg[:], rgf[:], g_all[:], AL.mult)
                # gpsimd-owned members' o finishes last: schedule them at the end
                o_fin = pe.tile([128, L, INNER], bf16, tag="o_fin")
                tail_order = [x for x in range(L) if x not in GP_O] + GP_O
                for l in tail_order:
                    nc.vector.tensor_tensor(
                        o_fin[:, l].rearrange("p (h d) -> p h d", d=DH),
                        o_acc[:, l].rearrange("p (h d) -> p h d", d=DH),
                        rg[:, l, :, None].to_broadcast([128, H, DH]), AL.mult)
                for l in tail_order:
                    oT = pes.tile([128, IT, T], bf16, tag="oT")
                    for it in range(IT):
                        tps = ptp.tile([128, 128], bf16, tag="tp")
                        nc.tensor.transpose(
                            tps[:], o_fin[:, l, it * 128:(it + 1) * 128],
                            ident_b[:])
                        nc.scalar.copy(oT[:, it], tps[:])
                    pout = pes.tile([128, D], bf16, tag="pout")
                    for oc in range(2):
                        ps = pmm.tile([128, INNER], f32, tag="mm")
                        for it in range(IT):
                            nc.tensor.matmul(ps[:], oT[:, it],
                                             wout[:, it, oc * 512:(oc + 1) * 512],
                                             start=(it == 0), stop=(it == IT - 1))
                        nc.scalar.copy(pout[:, oc * 512:(oc + 1) * 512], ps[:])
                    nc.sync.dma_start(out_d[l][:], pout[:])

    nc.compile()
    return nc


def get_nc(has_bias=False):
    if has_bias not in _NC_CACHE:
        _NC_CACHE[has_bias] = _build(has_bias)
    return _NC_CACHE[has_bias]


def _feat_major(w):
    """[D, F] -> [128, DT_w, F] with the contraction dim on partitions."""
    dt_w = w.shape[0] // 128
    return np.ascontiguousarray(
        w.reshape(dt_w, 128, w.shape[1]).transpose(1, 0, 2))


def prep_weights(w_net, b_net, norm_w, wq, wkv, knorm_w, wg, wout):
    import ml_dtypes
    bf = ml_dtypes.bfloat16

    wf = np.matmul(w_net.transpose(0, 2, 1), wkv[None])          # [L, D, 2I]
    colscale = (np.tile(knorm_w, H) * SCALE).astype(np.float32)
    wq2 = norm_w[:, None] * wq * colscale[None, :]
    out = dict(
        wkvk=_feat_major(wkv[:, :INNER]).astype(bf),
        wkvv=_feat_major(wkv[:, INNER:]).astype(bf),
        wfk=np.stack([_feat_major(wf[l, :, :INNER]) for l in range(L)])
        .astype(bf),
        wfv=np.stack([_feat_major(wf[l, :, INNER:]) for l in range(L)])
        .astype(bf),
        wqT=_feat_major(wq2).astype(bf),
        wgT=_feat_major(norm_w[:, None] * wg).astype(bf),
        woutT=_feat_major(wout).astype(bf),
        bkv=np.ascontiguousarray((b_net @ wkv)[None]).astype(bf),
    )
    return out


def prep_core_x(tokens, c):
    import ml_dtypes
    xs = tokens[:, :, c * NSL:(c + 1) * NSL, :].reshape(L, T, D)
    xT = xs.reshape(L, T, DT, 128).transpose(3, 2, 0, 1).reshape(128, DT, LT)
    return np.ascontiguousarray(xT).astype(ml_dtypes.bfloat16)


def make_in_maps(tokens, w_net, b_net, norm_w, wq, wkv, knorm_w, wg, wout):
    shared = prep_weights(np.asarray(w_net, np.float32), np.asarray(b_net, np.float32),
                          np.asarray(norm_w, np.float32), np.asarray(wq, np.float32),
                          np.asarray(wkv, np.float32), np.asarray(knorm_w, np.float32),
                          np.asarray(wg, np.float32), np.asarray(wout, np.float32))
    tokens = np.asarray(tokens, np.float32)
    return [dict(shared, xTb=prep_core_x(tokens, c)) for c in range(NCORES)]


def stitch(results):
    full = np.empty((L, B, N, D), dtype=np.float32)
    for c in range(NCORES):
        full[:, :, c * NSL:(c + 1) * NSL, :] = \
            results[c]["out"].astype(np.float32).reshape(L, B, NSL, D)
    return full


def kernel(tokens, w_net, b_net, norm_w, wq, wkv, knorm_w, wg, wout):
    nc = get_nc(has_bias=bool(np.any(np.asarray(b_net))))
    in_maps = make_in_maps(tokens, w_net, b_net, norm_w, wq, wkv, knorm_w, wg, wout)
    res = bass_utils.run_bass_kernel_spmd(nc, in_maps, core_ids=list(range(NCORES)))
    return stitch(res.results)


# revision 33
# speedup vs baseline: 1.5422x; 1.0057x over previous
"""Trainium2 Bass kernel for nn_EnsemblesWithMessagePassing.

Sharding: data-parallel over token positions (N=512 across 8 cores, 64
positions x B=2 = 128 (b,n) tokens per core). The voting attention is
per-position over the M=16 messages, so no collectives are needed; each
core holds all members' weights and computes all 8 members for its slice.

Structural choices (engine-balance driven; DVE is the bottleneck):
  * The member Linear feeds ONLY the kv projection, so (x@Wl^T)@wkv is
    fused on CPU into x@(Wl^T@wkv): no Linear matmuls, no transposes,
    no bias rank-1s, same DMA bytes.
  * Messages are stored m-interleaved (2l = token msg, 2l+1 = Linear-out
    msg of member l) so each member's two sims run as ONE product op +
    ONE 2x add-tree (tensor_reduce is 1x-only on DVE; tensor_tensor
    bf16 runs 2x).
  * Per member the PE emits k_tok, k_out, v_tok, v_out (k's first) so
    k copies/sims never wait behind v copies; k/v weight halves are
    separate DRAM tensors so the k half of a member's fused weight
    lands first (contiguous 8KB/partition rows, 128 descriptors).
  * Softmax skips max-subtraction (logits bounded ~|3.5|), which makes
    exp(logits) computable per m-half: the o-contribution of messages
    0..7 overlaps the second half of the kv loop, halving the DVE tail.
  * q stays RAW; the rms scale rides the combined logit scale
    rk[t,l,h,m] = rs[t,l]*krinv[t,h,m]. Gates fold into the softmax
    normalizer and multiply o once per member.
  * Everything bf16 except logits/stats; output written bf16, upcast on
    CPU.

b_net is zero in the reference setup; a bias-capable variant (rank-1
matmuls of b@wkv into the out-message PSUM groups) is compiled only if
a nonzero b_net shows up.
"""
import sys

for _p in ("/opt/trn_rl_repo", "/root/.axon_site/_ro/trn_rl_repo"):
    if _p not in sys.path:
        sys.path.insert(0, _p)

try:  # NTFF profile hook glue (only needed if BASS_TRACE is set externally)
    import antenv.axon_hooks  # noqa: F401
except Exception:
    pass

from contextlib import ExitStack

import numpy as np

import concourse.tile as tile
from concourse import bacc, mybir
from concourse import bass_utils
from concourse.masks import make_identity

f32 = mybir.dt.float32
bf16 = mybir.dt.bfloat16
AF = mybir.ActivationFunctionType
AL = mybir.AluOpType
AX = mybir.AxisListType

# problem shape
L, B, N, D = 8, 2, 512, 1024
H, DH = 8, 64
INNER = H * DH          # 512
M = 2 * L               # 16 messages
SCALE = DH ** -0.5
EPS = float(np.finfo(np.float32).eps)

NCORES = 8
NSL = N // NCORES       # 64 positions per core per batch row
T = B * NSL             # 128 tokens per core
LT = L * T              # 1024
DT = D // 128           # 8 d-tiles
IT = INNER // 128       # 4 inner-tiles
MH = M // 2             # half the messages (per softmax-split half)

GP_SIM = ()             # members whose sim pair runs on GPSIMD (measured: the
GP_O = []               # offload inflates DVE op times — keep everything on DVE)

_NC_CACHE = {}


def _build(has_bias=False):
    nc = bacc.Bacc("TRN2", target_bir_lowering=False, debug=False,
                   enable_asserts=False, num_devices=NCORES)

    xTb_d = nc.dram_tensor("xTb", [128, DT, LT], bf16, kind="ExternalInput").ap()
    wkvk_d = nc.dram_tensor("wkvk", [128, DT, INNER], bf16,
                            kind="ExternalInput").ap()
    wkvv_d = nc.dram_tensor("wkvv", [128, DT, INNER], bf16,
                            kind="ExternalInput").ap()
    wfk_d = nc.dram_tensor("wfk", [L, 128, DT, INNER], bf16,
                           kind="ExternalInput").ap()
    wfv_d = nc.dram_tensor("wfv", [L, 128, DT, INNER], bf16,
                           kind="ExternalInput").ap()
    wq_d = nc.dram_tensor("wqT", [128, DT, INNER], bf16, kind="ExternalInput").ap()
    wg_d = nc.dram_tensor("wgT", [128, DT, H], bf16, kind="ExternalInput").ap()
    wout_d = nc.dram_tensor("woutT", [128, IT, D], bf16, kind="ExternalInput").ap()
    bkv_d = nc.dram_tensor("bkv", [1, L, 2 * INNER], bf16,
                           kind="ExternalInput").ap()
    out_d = nc.dram_tensor("out", [L, T, D], bf16, kind="ExternalOutput").ap()

    with tile.TileContext(nc) as tc, ExitStack() as ctx:
        pc = ctx.enter_context(tc.tile_pool(name="const", bufs=1))
        pa = ctx.enter_context(tc.tile_pool(name="attp", bufs=1))
        po = ctx.enter_context(tc.tile_pool(name="oscr", bufs=2))
        pg = ctx.enter_context(tc.tile_pool(name="gscr", bufs=1))
        pmm = ctx.enter_context(tc.tile_pool(name="psmm", bufs=6, space="PSUM"))

        # ---- constants ----
        ident_b = pc.tile([128, 128], bf16, tag="ident_b")
        make_identity(nc, ident_b)
        ones_f = pc.tile([1, 2], f32, tag="ones_f")
        nc.vector.memset(ones_f[:], 1.0)
        onesc = pc.tile([128, 2], bf16, tag="onesc")
        nc.vector.memset(onesc[:], 1.0)
        eps_c = pc.tile([128, 1], f32, tag="eps")
        nc.vector.memset(eps_c[:], EPS)
        if has_bias:
            onesb = pc.tile([1, 128], bf16, tag="onesb")
            nc.vector.memset(onesb[:], 1.0)
            bkv = pc.tile([1, L, 2 * INNER], bf16, tag="bkv")
            nc.sync.dma_start(bkv[:], bkv_d[:])

        # whole-kernel attention state (m-interleaved: 2l=token, 2l+1=out)
        k_all = pa.tile([128, M, INNER], bf16, tag="k_all")     # RAW k
        v_allT = pa.tile([128, H, DH, M], bf16, tag="v_allT")   # m innermost
        q_all = pa.tile([128, L, INNER], bf16, tag="q_all")     # RAW q (no rs)
        g_all = pa.tile([128, L, H], f32, tag="g_all")
        kss = pa.tile([128, M, H], f32, tag="kss")              # m-major
        krinv = pa.tile([128, M, H], f32, tag="krinv")
        sim_all = pa.tile([128, L, H, M], f32, tag="sim_all")   # RAW q.k
        pl_all = pa.tile([128, L, H, M], bf16, tag="pl_all")
        o_acc = pa.tile([128, L, INNER], bf16, tag="o_acc")
        rs_tok = pa.tile([128, L], f32, tag="rs_tok")
        rk = pa.tile([128, L, H, M], f32, tag="rk")

        with ExitStack() as ctx_b:
            pb = ctx_b.enter_context(tc.tile_pool(name="bigact", bufs=1))
            ps_ = ctx_b.enter_context(tc.tile_pool(name="scr", bufs=2))
            psm = ctx_b.enter_context(tc.tile_pool(name="pssm", bufs=2,
                                                   space="PSUM"))

            xTb = pb.tile([128, DT, LT], bf16, tag="xTb")
            nc.sync.dma_start(xTb[:], xTb_d[:])
            wq = pb.tile([128, DT, INNER], bf16, tag="wq")
            nc.sync.dma_start(wq[:], wq_d[:])
            wkvk = pb.tile([128, DT, INNER], bf16, tag="wkvk")
            nc.sync.dma_start(wkvk[:], wkvk_d[:])
            wkvv = pb.tile([128, DT, INNER], bf16, tag="wkvv")
            nc.sync.dma_start(wkvv[:], wkvv_d[:])
            wg = pb.tile([128, DT, H], bf16, tag="wg")
            nc.sync.dma_start(wg[:], wg_d[:])

            def mm_sweep(ps, wsrc, l, nrows=INNER, extra=None):
                for d in range(DT):
                    nc.tensor.matmul(ps[:, 0:nrows], xTb[:, d, l * T:(l + 1) * T],
                                     wsrc[:, d, 0:nrows], start=(d == 0),
                                     stop=(d == DT - 1) and extra is None)
                if extra is not None:
                    nc.tensor.matmul(ps[:, 0:nrows], onesb[0:1, :], extra,
                                     start=False, stop=True)

            # ---- RAW queries first: the sim pipeline must start early ----
            with nc.named_scope("qg"):
                for l in range(L):
                    qps = pmm.tile([128, INNER], f32, tag="mm")
                    mm_sweep(qps, wq, l)
                    nc.scalar.copy(q_all[:, l], qps[:])

            def k_post(m):
                ksq = pg.tile([128, INNER], f32, tag="ksq")
                nc.scalar.square(ksq[:], k_all[:, m])
                nc.vector.tensor_reduce(
                    kss[:, m], ksq.rearrange("p (h d) -> p h d", d=DH),
                    axis=AX.X, op=AL.add)

            # ---- all token-message k's next (only need wkvk): the first
            # sims then wait only on the member's fused k weight DMA ----
            with nc.named_scope("ktok"):
                for l in range(L):
                    kt = pmm.tile([128, INNER], f32, tag="mm")
                    mm_sweep(kt, wkvk, l)
                    nc.scalar.copy(k_all[:, 2 * l], kt[:])
                    k_post(2 * l)

            # ---- token rms stats (token-major; rs folds into logits) ----
            with nc.named_scope("stats"), ExitStack() as ctx_s:
                pst = ctx_s.enter_context(tc.tile_pool(name="statp", bufs=1))
                sq = pst.tile([128, DT, LT], bf16, tag="sq")
                nc.vector.tensor_tensor(
                    sq.rearrange("p d t -> p (d t)"),
                    xTb.rearrange("p d t -> p (d t)"),
                    xTb.rearrange("p d t -> p (d t)"), AL.mult)
                ssq_row = pst.tile([1, LT], f32, tag="ssq_row")
                for c in range(2):
                    ssps = psm.tile([2, 512], f32, tag="sm")
                    for d in range(DT):
                        nc.tensor.matmul(ssps[:], onesc[:],
                                         sq[:, d, c * 512:(c + 1) * 512],
                                         start=(d == 0), stop=(d == DT - 1))
                    nc.scalar.copy(ssq_row[0:1, c * 512:(c + 1) * 512],
                                   ssps[0:1, :])
                rsps = psm.tile([128, L], f32, tag="sm")
                for l in range(L):
                    nc.tensor.matmul(rsps[:, l:l + 1],
                                     ssq_row[0:1, l * T:(l + 1) * T],
                                     ones_f[0:1, 0:1], start=True, stop=True)
                rms_tok = pst.tile([128, L], f32, tag="rms_tok")
                nc.scalar.activation(rms_tok[:], rsps[:], AF.Sqrt,
                                     scale=1.0 / D, bias=eps_c[:, 0:1])
                nc.vector.reciprocal(rs_tok[:], rms_tok[:])

            def sim_pair(l, eng):
                # both messages of member l in one product + one 2x add-tree;
                # eng is nc.vector or nc.gpsimd (DVE offload)
                pool = ps_ if eng is nc.vector else pg
                scr = pool.tile([128, L, 2, INNER], bf16,
                                tag="scr" if eng is nc.vector else "scrg")
                m0 = 2 * l
                eng.tensor_tensor(
                    scr[:],
                    q_all[:, :, None].to_broadcast([128, L, 2, INNER]),
                    k_all[:, None, m0:m0 + 2].to_broadcast([128, L, 2, INNER]),
                    AL.mult)
                sv = scr.rearrange("p l m (h d) -> p (l m) h d", d=DH)
                w = DH // 2
                while w >= 1:
                    eng.tensor_tensor(sv[:, :, :, 0:w], sv[:, :, :, 0:w],
                                      sv[:, :, :, w:2 * w], AL.add)
                    w //= 2
                # deposit [p, l, m2, h] into sim_all's [p, l, h, m] layout
                eng.tensor_copy(
                    sim_all[:, :, :, m0:m0 + 2].rearrange("p l h m -> p l m h"),
                    scr.rearrange("p l m (h d) -> p l m h d", d=DH)[:, :, :, :, 0])

            def half_softmax_o(half):
                # exp over messages [half*MH, (half+1)*MH) and their o part
                ms = slice(half * MH, (half + 1) * MH)
                krm = po.tile([128, MH, H], f32, tag="krm")
                nc.scalar.activation(krm.rearrange("p m h -> p (m h)"),
                                     kss[:, ms].rearrange("p m h -> p (m h)"),
                                     AF.Sqrt, scale=1.0 / DH, bias=eps_c[:, 0:1])
                nc.vector.reciprocal(
                    krinv[:, ms].rearrange("p m h -> p (m h)"),
                    krm.rearrange("p m h -> p (m h)"))
                nc.vector.tensor_tensor(
                    rk[:, :, :, ms],
                    rs_tok[:, :, None, None].to_broadcast([128, L, H, MH]),
                    krinv[:, ms].rearrange("p m h -> p h m")[:, None]
                    .to_broadcast([128, L, H, MH]), AL.mult)
                nc.vector.tensor_tensor(sim_all[:, :, :, ms], sim_all[:, :, :, ms],
                                        rk[:, :, :, ms], AL.mult)
                nc.scalar.activation(
                    pl_all[:, :, :, ms].rearrange("p l h m -> p (l h) m"),
                    sim_all[:, :, :, ms].rearrange("p l h m -> p (l h) m"),
                    AF.Exp)
                for l in GP_O + [x for x in range(L) if x not in GP_O]:
                    eng = nc.gpsimd if l in GP_O else nc.vector
                    prod = (pg if l in GP_O else po).tile(
                        [128, H, DH, MH], bf16,
                        tag="prodg" if l in GP_O else "prod")
                    eng.tensor_tensor(
                        prod[:],
                        pl_all[:, l, :, None, ms].to_broadcast([128, H, DH, MH]),
                        v_allT[:, :, :, ms], AL.mult)
                    for w in (MH // 2, MH // 4):
                        eng.tensor_tensor(
                            prod[:, :, :, 0:w], prod[:, :, :, 0:w],
                            prod[:, :, :, w:2 * w], AL.add)
                    ov = o_acc[:, l].rearrange("p (h d) -> p h d", d=DH)
                    if half == 0:
                        eng.tensor_tensor(ov, prod[:, :, :, 0],
                                          prod[:, :, :, 1], AL.add)
                    else:
                        eng.tensor_tensor(prod[:, :, :, 0], prod[:, :, :, 0],
                                          prod[:, :, :, 1], AL.add)
                        eng.tensor_tensor(ov, ov, prod[:, :, :, 0], AL.add)

            # ---- kv + sims, with first-half softmax/o overlapped ----
            with ExitStack() as ctx_l, nc.named_scope("linkv"):
                plw = ctx_l.enter_context(tc.tile_pool(name="linw", bufs=2))
                wfk_t, wfv_t = [], []

                def fetch(j):
                    tk = plw.tile([128, DT, INNER], bf16, tag="wfk")
                    nc.sync.dma_start(tk[:], wfk_d[j])
                    tv = plw.tile([128, DT, INNER], bf16, tag="wfv")
                    nc.sync.dma_start(tv[:], wfv_d[j])
                    wfk_t.append(tk)
                    wfv_t.append(tv)

                fetch(0)
                fetch(1)
                for l in range(L):
                    m0 = 2 * l
                    bk = bv = None
                    if has_bias:
                        bk = bkv[0:1, l, 0:INNER]
                        bv = bkv[0:1, l, INNER:2 * INNER]
                    ko = pmm.tile([128, INNER], f32, tag="mm")
                    mm_sweep(ko, wfk_t[l], l, extra=bk)
                    nc.scalar.copy(k_all[:, m0 + 1], ko[:])
                    k_post(m0 + 1)
                    sim_pair(l, nc.gpsimd if l in GP_SIM else nc.vector)
                    vt = pmm.tile([128, INNER], f32, tag="mm")
                    mm_sweep(vt, wkvv, l)
                    nc.scalar.copy(v_allT[:, :, :, m0],
                                   vt.rearrange("p (h d) -> p h d", d=DH))
                    vo = pmm.tile([128, INNER], f32, tag="mm")
                    mm_sweep(vo, wfv_t[l], l, extra=bv)
                    nc.scalar.copy(v_allT[:, :, :, m0 + 1],
                                   vo.rearrange("p (h d) -> p h d", d=DH))
                    if l + 2 < L:
                        fetch(l + 2)
                    if l == L // 2 - 1:
                        half_softmax_o(0)   # overlaps members 4..7

                # gates last (PE is free here; g needed only for rg below)
                for l in range(L):
                    gps = psm.tile([128, L], f32, tag="sm")
                    mm_sweep(gps, wg, l, nrows=H)
                    nc.scalar.activation(g_all[:, l], gps[:, 0:H], AF.Sigmoid,
                                         scale=rs_tok[:, l:l + 1])

        # ---- second softmax half + normalize + pooled ----
        with ExitStack() as ctx_e:
            pe = ctx_e.enter_context(tc.tile_pool(name="outp", bufs=1))
            pes = ctx_e.enter_context(tc.tile_pool(name="outs", bufs=2))
            ptp = ctx_e.enter_context(tc.tile_pool(name="pstp", bufs=2,
                                                   space="PSUM"))
            wout = pe.tile([128, IT, D], bf16, tag="wout")
            nc.sync.dma_start(wout[:], wout_d[:])
            with nc.named_scope("attn"):
                half_softmax_o(1)
                sm_all = pe.tile([128, L, H], f32, tag="sm_all")
                nc.vector.tensor_reduce(sm_all[:], pl_all[:], axis=AX.X, op=AL.add)
                rgf = pe.tile([128, L, H], f32, tag="rgf")
                nc.vector.reciprocal(rgf.rearrange("p l h -> p (l h)"),
                                     sm_all.rearrange("p l h -> p (l h)"))
                rg = pe.tile([128, L, H], bf16, tag="rg")
                nc.vector.tensor_tensor(rg[:], rgf[:], g_all[:], AL.mult)
                # gpsimd-owned members' o finishes last: schedule them at the end
                o_fin = pe.tile([128, L, INNER], bf16, tag="o_fin")
                tail_order = [x for x in range(L) if x not in GP_O] + GP_O
                for l in tail_order:
                    nc.vector.tensor_tensor(
                        o_fin[:, l].rearrange("p (h d) -> p h d", d=DH),
                        o_acc[:, l].rearrange("p (h d) -> p h d", d=DH),
                        rg[:, l, :, None].to_broadcast([128, H, DH]), AL.mult)
                for l in tail_order:
                    oT = pes.tile([128, IT, T], bf16, tag="oT")
                    for it in range(IT):
                        tps = ptp.tile([128, 128], bf16, tag="tp")
                        nc.tensor.transpose(
                            tps[:], o_fin[:, l, it * 128:(it + 1) * 128],
                            ident_b[:])
                        nc.scalar.copy(oT[:, it], tps[:])
                    pout = pes.tile([128, D], bf16, tag="pout")
                    for oc in range(2):
                        ps = pmm.tile([128, INNER], f32, tag="mm")
                        for it in range(IT):
                            nc.tensor.matmul(ps[:], oT[:, it],
                                             wout[:, it, oc * 512:(oc + 1) * 512],
                                             start=(it == 0), stop=(it == IT - 1))
                        nc.scalar.copy(pout[:, oc * 512:(oc + 1) * 512], ps[:])
                    nc.sync.dma_start(out_d[l][:], pout[:])

    nc.compile()
    return nc


def get_nc(has_bias=False):
    if has_bias not in _NC_CACHE:
        _NC_CACHE[has_bias] = _build(has_bias)
    return _NC_CACHE[has_bias]


def _feat_major(w):
    """[D, F] -> [128, DT_w, F] with the contraction dim on partitions."""
    dt_w = w.shape[0] // 128
    return np.ascontiguousarray(
        w.reshape(dt_w, 128, w.shape[1]).transpose(1, 0, 2))


def prep_weights(w_net, b_net, norm_w, wq, wkv, knorm_w, wg, wout):
    import ml_dtypes
    bf = ml_dtypes.bfloat16

    wf = np.matmul(w_net.transpose(0, 2, 1), wkv[None])          # [L, D, 2I]
    colscale = (np.tile(knorm_w, H) * SCALE).astype(np.float32)
    wq2 = norm_w[:, None] * wq * colscale[None, :]
    out = dict(
        wkvk=_feat_major(wkv[:, :INNER]).astype(bf),
        wkvv=_feat_major(wkv[:, INNER:]).astype(bf),
        wfk=np.stack([_feat_major(wf[l, :, :INNER]) for l in range(L)])
        .astype(bf),
        wfv=np.stack([_feat_major(wf[l, :, INNER:]) for l in range(L)])
        .astype(bf),
        wqT=_feat_major(wq2).astype(bf),
        wgT=_feat_major(norm_w[:, None] * wg).astype(bf),
        woutT=_feat_major(wout).astype(bf),
        bkv=np.ascontiguousarray((b_net @ wkv)[None]).astype(bf),
    )
    return out


def prep_core_x(tokens, c):
    import ml_dtypes
    xs = tokens[:, :, c * NSL:(c + 1) * NSL, :].reshape(L, T, D)
    xT = xs.reshape(L, T, DT, 128).transpose(3, 2, 0, 1).reshape(128, DT, LT)
    return np.ascontiguousarray(xT).astype(ml_dtypes.bfloat16)


def make_in_maps(tokens, w_net, b_net, norm_w, wq, wkv, knorm_w, wg, wout):
    shared = prep_weights(np.asarray(w_net, np.float32), np.asarray(b_net, np.float32),
                          np.asarray(norm_w, np.float32), np.asarray(wq, np.float32),
                          np.asarray(wkv, np.float32), np.asarray(knorm_w, np.float32),
                          np.asarray(wg, np.float32), np.asarray(wout, np.float32))
    tokens = np.asarray(tokens, np.float32)
    return [dict(shared, xTb=prep_core_x(tokens, c)) for c in range(NCORES)]


def stitch(results):
    full = np.empty((L, B, N, D), dtype=np.float32)
    for c in range(NCORES):
        full[:, :, c * NSL:(c + 1) * NSL, :] = \
            results[c]["out"].astype(np.float32).reshape(L, B, NSL, D)
    return full


def kernel(tokens, w_net, b_net, norm_w, wq, wkv, knorm_w, wg, wout):
    nc = get_nc(has_bias=bool(np.any(np.asarray(b_net))))
    in_maps = make_in_maps(tokens, w_net, b_net, norm_w, wq, wkv, knorm_w, wg, wout)
    res = bass_utils.run_bass_kernel_spmd(nc, in_maps, core_ids=list(range(NCORES)))
    return stitch(res.results)
